# revision 7
# baseline (speedup 1.0000x reference)
"""Trainium2 Bass kernel for nn_ConditionalControlModule (histogram_binning).

Per frame (512x512 RGB): RGB -> HSV -> per-channel 32-bin value histogram +
256-bin LBP histogram. The device computes per-(frame,channel) joint
histograms via one-hot nibble encodings contracted on the tensor engine
(4 x-columns per matmul into PSUM). The host does the tiny projector math.

Sharding: 32 frames split 4-per-core across 8 NeuronCores (data parallel;
host combines per-core partial results).
"""
import sys
sys.path.insert(0, '/opt/trn_rl_repo')
import os
import numpy as np

import concourse.bacc as bacc
import concourse.tile as tile
from concourse import mybir
from concourse.bass_utils import run_bass_kernel_spmd

F32 = mybir.dt.float32
BF16 = mybir.dt.bfloat16
I32 = mybir.dt.int32
U8 = mybir.dt.uint8
AL = mybir.AluOpType
ACTF = mybir.ActivationFunctionType

P = 128
H = 512
W = 512
WT = W + 2       # tile width with x halos
NSLAB = 4
G = 4            # x-columns merged per matmul
XG = W // G
ML = 128         # lhsT columns: 4*24 one-hot + 32 zero pad (FWL-friendly)
NR = 80          # rhs columns: 4*20
C1M = float(np.float32(1.0 - 2.0 ** -24))

# LBP neighbor (dy, dx) per bit, from reference offsets minus pad center
LBP_OFFS = [(-1, -1), (-1, 0), (-1, 1), (0, 1), (1, 1), (1, 0), (1, -1), (0, -1)]


def _build_nc(nframes):
    nc = bacc.Bacc("TRN2", num_devices=8)
    seq = nc.dram_tensor("seq", [nframes, 3, H, W], F32, kind="ExternalInput")
    nfc = nframes * 3
    out = nc.dram_tensor("out", [nfc, P, NR], F32, kind="ExternalOutput")

    with tile.TileContext(nc) as tc:
        with tc.tile_pool(name="cst", bufs=1) as cst, \
             tc.tile_pool(name="rgb", bufs=4) as rgbp, \
             tc.tile_pool(name="hsvp", bufs=15) as hsvp, \
             tc.tile_pool(name="sht", bufs=2) as shtp, \
             tc.tile_pool(name="wrk", bufs=8) as wrk, \
             tc.tile_pool(name="bitp", bufs=10) as bitp, \
             tc.tile_pool(name="nibp", bufs=5) as nibp, \
             tc.tile_pool(name="wi", bufs=2) as wip, \
             tc.tile_pool(name="msk", bufs=3) as mskp, \
             tc.tile_pool(name="ob", bufs=2) as obp, \
             tc.tile_pool(name="ps", bufs=2, space="PSUM") as ps:

            def iota_bf(n, name):
                ti = cst.tile([P, n], I32, tag=f"ioi_{name}")
                nc.gpsimd.iota(ti[:], pattern=[[1, n]], base=0, channel_multiplier=0)
                tb = cst.tile([P, n], BF16, tag=f"iob_{name}")
                nc.vector.tensor_copy(out=tb[:], in_=ti[:])
                return tb

            iota16 = iota_bf(16, "16")
            iota8 = iota_bf(8, "8")
            iota4 = iota_bf(4, "4")
            zeros = cst.tile([P, WT], F32, tag="zeros")
            nc.gpsimd.memset(zeros[:], 0.0)
            ones = cst.tile([P, WT], F32, tag="ones")
            nc.gpsimd.memset(ones[:], 1.0)
            bias2 = cst.tile([P, 1], F32, tag="bias2")
            nc.gpsimd.memset(bias2[:], 2.0)
            bias4 = cst.tile([P, 1], F32, tag="bias4")
            nc.gpsimd.memset(bias4[:], 4.0)

            # manual double-buffered one-hot tensors; ohL zero pad written once
            ohL_bufs, ohR_bufs = [], []
            for i in range(2):
                tL = cst.tile([P, XG, ML], BF16, tag=f"ohL{i}")
                nc.gpsimd.memset(tL[:, :, 4 * 24:], 0.0)
                ohL_bufs.append(tL)
                tR = cst.tile([P, XG, NR], BF16, tag=f"ohR{i}")
                ohR_bufs.append(tR)

            slabch_idx = 0

            for f in range(nframes):
                # ---- RGB -> HSV for all slabs of this frame ----
                hsv_tiles = [[None] * 3 for _ in range(NSLAB)]
                for s in range(NSLAB):
                    r0 = s * P
                    rgb = []
                    for c in range(3):
                        t = rgbp.tile([P, WT], F32, tag="rgb")
                        nc.sync.dma_start(out=t[:, 1:513], in_=seq[f, c, r0:r0 + P, :])
                        nc.gpsimd.tensor_copy(out=t[:, 0:1], in_=t[:, 2:3])
                        nc.gpsimd.tensor_copy(out=t[:, 513:514], in_=t[:, 511:512])
                        rgb.append(t)
                    r, g, b = rgb

                    maxc = hsvp.tile([P, WT], F32, tag="hsv")  # = V channel
                    nc.vector.tensor_tensor(out=maxc[:], in0=r[:], in1=g[:], op=AL.max)
                    nc.vector.tensor_tensor(out=maxc[:], in0=maxc[:], in1=b[:], op=AL.max)
                    minc = wrk.tile([P, WT], F32, tag="w")
                    nc.vector.tensor_tensor(out=minc[:], in0=r[:], in1=g[:], op=AL.min)
                    nc.vector.tensor_tensor(out=minc[:], in0=minc[:], in1=b[:], op=AL.min)
                    delta = wrk.tile([P, WT], F32, tag="w")
                    nc.vector.tensor_tensor(out=delta[:], in0=maxc[:], in1=minc[:], op=AL.subtract)
                    mask0 = mskp.tile([P, WT], U8, tag="m")  # delta <= 1e-6
                    nc.vector.tensor_scalar(out=mask0[:], in0=delta[:], scalar1=1e-6,
                                            scalar2=None, op0=AL.is_le)
                    inv = wrk.tile([P, WT], F32, tag="w")
                    nc.vector.select(out=inv[:], mask=mask0[:], on_true=ones[:], on_false=delta[:])
                    nc.vector.reciprocal(out=inv[:], in_=inv[:])

                    hue = hsvp.tile([P, WT], F32, tag="hsv")
                    nc.gpsimd.memset(hue[:], 0.0)
                    tdif = wrk.tile([P, WT], F32, tag="w")
                    tval = wrk.tile([P, WT], F32, tag="w")
                    meq = mskp.tile([P, WT], U8, tag="m")
                    # r-branch: mod((g-b)*inv, 6)
                    nc.vector.tensor_tensor(out=tdif[:], in0=g[:], in1=b[:], op=AL.subtract)
                    nc.vector.tensor_tensor(out=tdif[:], in0=tdif[:], in1=inv[:], op=AL.mult)
                    tneg = wrk.tile([P, WT], F32, tag="w")
                    nc.vector.tensor_scalar(out=tneg[:], in0=tdif[:], scalar1=0.0,
                                            scalar2=None, op0=AL.is_lt)
                    nc.vector.scalar_tensor_tensor(out=tval[:], in0=tneg[:], scalar=6.0,
                                                   in1=tdif[:], op0=AL.mult, op1=AL.add)
                    nc.vector.tensor_tensor(out=meq[:], in0=maxc[:], in1=r[:], op=AL.is_equal)
                    nc.vector.copy_predicated(out=hue[:], mask=meq[:], data=tval[:])
                    # g-branch: (b-r)*inv + 2
                    nc.vector.tensor_tensor(out=tdif[:], in0=b[:], in1=r[:], op=AL.subtract)
                    nc.vector.tensor_tensor(out=tdif[:], in0=tdif[:], in1=inv[:], op=AL.mult)
                    nc.scalar.activation(out=tval[:], in_=tdif[:], func=ACTF.Identity, bias=bias2[:])
                    nc.vector.tensor_tensor(out=meq[:], in0=maxc[:], in1=g[:], op=AL.is_equal)
                    nc.vector.copy_predicated(out=hue[:], mask=meq[:], data=tval[:])
                    # b-branch: (r-g)*inv + 4
                    nc.vector.tensor_tensor(out=tdif[:], in0=r[:], in1=g[:], op=AL.subtract)
                    nc.vector.tensor_tensor(out=tdif[:], in0=tdif[:], in1=inv[:], op=AL.mult)
                    nc.scalar.activation(out=tval[:], in_=tdif[:], func=ACTF.Identity, bias=bias4[:])
                    nc.vector.tensor_tensor(out=meq[:], in0=maxc[:], in1=b[:], op=AL.is_equal)
                    nc.vector.copy_predicated(out=hue[:], mask=meq[:], data=tval[:])
                    # zero where delta<=1e-6, then /6
                    nc.vector.copy_predicated(out=hue[:], mask=mask0[:], data=zeros[:])
                    nc.scalar.activation(out=hue[:], in_=hue[:], func=ACTF.Copy,
                                         scale=float(np.float32(1.0 / 6.0)))

                    # sat
                    sat = hsvp.tile([P, WT], F32, tag="hsv")
                    mx = wrk.tile([P, WT], F32, tag="w")
                    nc.vector.tensor_scalar(out=mx[:], in0=maxc[:], scalar1=1e-6,
                                            scalar2=None, op0=AL.max)
                    nc.vector.reciprocal(out=mx[:], in_=mx[:])
                    nc.vector.tensor_tensor(out=sat[:], in0=delta[:], in1=mx[:], op=AL.mult)
                    msat = mskp.tile([P, WT], U8, tag="m")
                    nc.vector.tensor_scalar(out=msat[:], in0=maxc[:], scalar1=1e-6,
                                            scalar2=None, op0=AL.is_le)
                    nc.vector.copy_predicated(out=sat[:], mask=msat[:], data=zeros[:])

                    hsv_tiles[s] = [hue, sat, maxc]

                # ---- histograms per (channel, slab) ----
                for c in range(3):
                    fc = f * 3 + c
                    pm = ps.tile([P, NR], F32, space="PSUM", tag="pm")
                    for s in range(NSLAB):
                        ct = hsv_tiles[s][c]
                        up = shtp.tile([P, WT], F32, tag="up")
                        if s == 0:
                            nc.sync.dma_start(out=up[0:1], in_=hsv_tiles[0][c][1:2])
                        else:
                            nc.sync.dma_start(out=up[0:1], in_=hsv_tiles[s - 1][c][127:128])
                        nc.sync.dma_start(out=up[1:128], in_=ct[0:127])
                        dn = shtp.tile([P, WT], F32, tag="dn")
                        nc.sync.dma_start(out=dn[0:127], in_=ct[1:128])
                        if s == NSLAB - 1:
                            nc.sync.dma_start(out=dn[127:128], in_=hsv_tiles[s][c][126:127])
                        else:
                            nc.sync.dma_start(out=dn[127:128], in_=hsv_tiles[s + 1][c][0:1])
                        rowt = {-1: up, 0: ct, 1: dn}

                        ohL = ohL_bufs[slabch_idx % 2]
                        ohR = ohR_bufs[slabch_idx % 2]
                        slabch_idx += 1

                        # LBP bits
                        bits = []
                        for k, (dy, dx) in enumerate(LBP_OFFS):
                            bt = bitp.tile([P, W], BF16, tag="bit")
                            eng = nc.vector
                            eng.tensor_tensor(out=bt[:],
                                              in0=rowt[dy][:, 1 + dx:513 + dx],
                                              in1=ct[:, 1:513], op=AL.is_ge)
                            bits.append(bt)
                        lonib = nibp.tile([P, W], BF16, tag="nib")
                        nc.vector.scalar_tensor_tensor(out=lonib[:], in0=bits[1][:], scalar=2.0,
                                                       in1=bits[0][:], op0=AL.mult, op1=AL.add)
                        nc.vector.scalar_tensor_tensor(out=lonib[:], in0=bits[2][:], scalar=4.0,
                                                       in1=lonib[:], op0=AL.mult, op1=AL.add)
                        nc.vector.scalar_tensor_tensor(out=lonib[:], in0=bits[3][:], scalar=8.0,
                                                       in1=lonib[:], op0=AL.mult, op1=AL.add)
                        hinib = nibp.tile([P, W], BF16, tag="nib")
                        nc.vector.scalar_tensor_tensor(out=hinib[:], in0=bits[5][:], scalar=2.0,
                                                       in1=bits[4][:], op0=AL.mult, op1=AL.add)
                        nc.vector.scalar_tensor_tensor(out=hinib[:], in0=bits[6][:], scalar=4.0,
                                                       in1=hinib[:], op0=AL.mult, op1=AL.add)
                        nc.vector.scalar_tensor_tensor(out=hinib[:], in0=bits[7][:], scalar=8.0,
                                                       in1=hinib[:], op0=AL.mult, op1=AL.add)

                        # HSV bin indices with exact floor
                        cm = wrk.tile([P, W], F32, tag="w")
                        nc.vector.tensor_scalar(out=cm[:], in0=ct[:, 1:513], scalar1=C1M,
                                                scalar2=None, op0=AL.min)
                        t32 = wrk.tile([P, W], F32, tag="w")
                        nc.scalar.activation(out=t32[:], in_=cm[:], func=ACTF.Copy, scale=32.0)
                        i32t = wip.tile([P, W], I32, tag="wi")
                        nc.vector.tensor_copy(out=i32t[:], in_=t32[:])
                        f32b = wrk.tile([P, W], F32, tag="w")
                        nc.vector.tensor_copy(out=f32b[:], in_=i32t[:])
                        gt = wrk.tile([P, W], F32, tag="w")
                        nc.vector.tensor_tensor(out=gt[:], in0=f32b[:], in1=t32[:], op=AL.is_gt)
                        idx = wrk.tile([P, W], F32, tag="w")
                        nc.vector.tensor_tensor(out=idx[:], in0=f32b[:], in1=gt[:], op=AL.subtract)
                        h8f = wrk.tile([P, W], F32, tag="w")
                        nc.scalar.activation(out=h8f[:], in_=idx[:], func=ACTF.Copy,
                                             scale=0.25, bias=-0.375)
                        h8i = wip.tile([P, W], I32, tag="wi")
                        nc.vector.tensor_copy(out=h8i[:], in_=h8f[:])
                        h8 = wrk.tile([P, W], F32, tag="w")
                        nc.vector.tensor_copy(out=h8[:], in_=h8i[:])
                        h8b = nibp.tile([P, W], BF16, tag="nib")
                        nc.gpsimd.tensor_copy(out=h8b[:], in_=h8[:])
                        lo4 = nibp.tile([P, W], BF16, tag="nib")
                        nc.vector.scalar_tensor_tensor(out=lo4[:], in0=h8[:], scalar=-4.0,
                                                       in1=idx[:], op0=AL.mult, op1=AL.add)

                        # one-hot writes, interleaved by g = x % G
                        dstL = ohL[:, :, 0:96].rearrange("p x (g t) -> p x g t", g=G)
                        dstR = ohR[:].rearrange("p x (g t) -> p x g t", g=G)
                        io16 = iota16[:].unsqueeze(1).unsqueeze(1).to_broadcast([P, XG, G, 16])
                        io8 = iota8[:].unsqueeze(1).unsqueeze(1).to_broadcast([P, XG, G, 8])
                        io4 = iota4[:].unsqueeze(1).unsqueeze(1).to_broadcast([P, XG, G, 4])

                        src = hinib[:].rearrange("p (x g) -> p x g", g=G).unsqueeze(3)
                        nc.vector.tensor_tensor(out=dstL[:, :, :, 0:16],
                                                in0=src.to_broadcast([P, XG, G, 16]),
                                                in1=io16, op=AL.is_equal)
                        src = h8b[:].rearrange("p (x g) -> p x g", g=G).unsqueeze(3)
                        nc.vector.tensor_tensor(out=dstL[:, :, :, 16:24],
                                                in0=src.to_broadcast([P, XG, G, 8]),
                                                in1=io8, op=AL.is_equal)
                        src = lonib[:].rearrange("p (x g) -> p x g", g=G).unsqueeze(3)
                        nc.vector.tensor_tensor(out=dstR[:, :, :, 0:16],
                                                in0=src.to_broadcast([P, XG, G, 16]),
                                                in1=io16, op=AL.is_equal)
                        src = lo4[:].rearrange("p (x g) -> p x g", g=G).unsqueeze(3)
                        nc.vector.tensor_tensor(out=dstR[:, :, :, 16:20],
                                                in0=src.to_broadcast([P, XG, G, 4]),
                                                in1=io4, op=AL.is_equal)

                        for xg in range(XG):
                            nc.tensor.matmul(
                                out=pm[:],
                                lhsT=ohL[:, xg, :],
                                rhs=ohR[:, xg, :],
                                start=(s == 0 and xg == 0),
                                stop=(s == NSLAB - 1 and xg == XG - 1),
                            )
                    ob = obp.tile([P, NR], F32, tag="ob")
                    nc.vector.tensor_copy(out=ob[:], in_=pm[:])
                    nc.sync.dma_start(out=out[fc], in_=ob[:])

    nc.finalize()
    return nc


_NC_CACHE = {}


def _get_nc(nframes):
    if nframes not in _NC_CACHE:
        _NC_CACHE[nframes] = _build_nc(nframes)
    return _NC_CACHE[nframes]


def device_histograms(frames, nframes_per_core=1, n_cores=8, **run_kwargs):
    """frames: [N, 3, 512, 512] f32. Returns (hsv_counts [N,3,32],
    lbp_counts [N,3,256], BassKernelResults)."""
    nc = _get_nc(nframes_per_core)
    in_maps = []
    for k in range(n_cores):
        sl = frames[k * nframes_per_core:(k + 1) * nframes_per_core]
        in_maps.append({"seq": np.ascontiguousarray(sl, dtype=np.float32)})
    res = run_bass_kernel_spmd(nc, in_maps, core_ids=list(range(n_cores)), **run_kwargs)
    N = nframes_per_core * n_cores
    hsv_counts = np.zeros((N, 3, 32), np.float32)
    lbp_counts = np.zeros((N, 3, 256), np.float32)
    for k in range(n_cores):
        pmall = res.results[k]["out"]
        for f in range(nframes_per_core):
            for c in range(3):
                pm = pmall[f * 3 + c]
                lbp = np.zeros((16, 16), np.float32)
                hsv = np.zeros((8, 4), np.float32)
                for g in range(G):
                    blk = pm[24 * g:24 * g + 24, 20 * g:20 * g + 20]
                    lbp += blk[0:16, 0:16]
                    hsv += blk[16:24, 16:20]
                n = k * nframes_per_core + f
                lbp_counts[n, c] = lbp.reshape(256)
                hsv_counts[n, c] = hsv.reshape(32)
    return hsv_counts, lbp_counts, res


def kernel(sequence, high, W, b, environment_id, season_id):
    sequence = np.asarray(sequence, dtype=np.float32)
    high = np.asarray(high, dtype=np.float32)
    Wp = np.asarray(W, dtype=np.float32)
    bp = np.asarray(b, dtype=np.float32)
    B, T, C, Hh, Ww = sequence.shape
    BT = B * T
    frames = sequence.reshape(BT, C, Hh, Ww)

    npc = 1
    per_batch = npc * 8
    hs, ls = [], []
    for b0 in range(0, BT, per_batch):
        h_, l_, _ = device_histograms(frames[b0:b0 + per_batch], nframes_per_core=npc)
        hs.append(h_)
        ls.append(l_)
    hsv_counts = np.concatenate(hs, axis=0)
    lbp_counts = np.concatenate(ls, axis=0)

    hist_feat = hsv_counts / np.maximum(hsv_counts.sum(-1, keepdims=True), 1.0)
    lbp_feat = lbp_counts / np.maximum(lbp_counts.sum(-1, keepdims=True), 1.0)
    low = np.concatenate([hist_feat.reshape(BT, 96), lbp_feat.reshape(BT, 768)],
                         axis=1).astype(np.float32)
    fused = np.concatenate([high, low], axis=1).reshape(B, T, -1)
    aggregated = fused.mean(axis=1, dtype=np.float32)
    z = (aggregated @ Wp.T + bp).astype(np.float32)
    c_feat = np.where(z >= 0, z, np.float32(0.2) * z).astype(np.float32)

    env_ids = np.asarray(environment_id).astype(np.int64)
    sea_ids = np.asarray(season_id).astype(np.int64)
    env = np.zeros((B, 4), np.float32)
    env[np.arange(B), env_ids] = 1.0
    sea = np.zeros((B, 4), np.float32)
    sea[np.arange(B), sea_ids] = 1.0
    c_cls = np.concatenate([env, sea], axis=1)
    return (c_feat, c_cls)


# revision 9
# speedup vs baseline: 1.0005x; 1.0005x over previous
"""Trainium2 Bass kernel for nn_ConditionalControlModule (histogram_binning).

Per frame (512x512 RGB): RGB -> HSV -> per-channel 32-bin value histogram +
256-bin LBP histogram. The device computes per-(frame,channel) joint
histograms via one-hot nibble encodings contracted on the tensor engine
(4 x-columns per matmul into PSUM). The host does the tiny projector math.

Sharding: 32 frames split 4-per-core across 8 NeuronCores (data parallel;
host combines per-core partial results).
"""
import sys
sys.path.insert(0, '/opt/trn_rl_repo')
import os
import numpy as np

import concourse.bacc as bacc
import concourse.tile as tile
from concourse import mybir
from concourse.bass_utils import run_bass_kernel_spmd

F32 = mybir.dt.float32
BF16 = mybir.dt.bfloat16
I32 = mybir.dt.int32
U8 = mybir.dt.uint8
AL = mybir.AluOpType
ACTF = mybir.ActivationFunctionType

P = 128
H = 512
W = 512
WT = W + 2       # tile width with x halos
NSLAB = 4
G = 4            # x-columns merged per matmul
XG = W // G
ML = 128         # lhsT columns: 4*24 one-hot + 32 zero pad (FWL-friendly)
NR = 80          # rhs columns: 4*20
C1M = float(np.float32(1.0 - 2.0 ** -24))

# LBP neighbor (dy, dx) per bit, from reference offsets minus pad center
LBP_OFFS = [(-1, -1), (-1, 0), (-1, 1), (0, 1), (1, 1), (1, 0), (1, -1), (0, -1)]


def _build_nc(nframes):
    nc = bacc.Bacc("TRN2", num_devices=8)
    seq = nc.dram_tensor("seq", [nframes, 3, H, W], F32, kind="ExternalInput")
    nfc = nframes * 3
    out = nc.dram_tensor("out", [nfc, P, NR], F32, kind="ExternalOutput")

    with tile.TileContext(nc) as tc:
        with tc.tile_pool(name="cst", bufs=1) as cst, \
             tc.tile_pool(name="rgb", bufs=4) as rgbp, \
             tc.tile_pool(name="hsvp", bufs=15) as hsvp, \
             tc.tile_pool(name="sht", bufs=2) as shtp, \
             tc.tile_pool(name="wrk", bufs=8) as wrk, \
             tc.tile_pool(name="bitp", bufs=10) as bitp, \
             tc.tile_pool(name="nibp", bufs=5) as nibp, \
             tc.tile_pool(name="wi", bufs=2) as wip, \
             tc.tile_pool(name="msk", bufs=3) as mskp, \
             tc.tile_pool(name="ob", bufs=2) as obp, \
             tc.tile_pool(name="ps", bufs=2, space="PSUM") as ps:

            def iota_bf(n, name):
                ti = cst.tile([P, n], I32, tag=f"ioi_{name}")
                nc.gpsimd.iota(ti[:], pattern=[[1, n]], base=0, channel_multiplier=0)
                tb = cst.tile([P, n], BF16, tag=f"iob_{name}")
                nc.vector.tensor_copy(out=tb[:], in_=ti[:])
                return tb

            iota16 = iota_bf(16, "16")
            iota8 = iota_bf(8, "8")
            iota4 = iota_bf(4, "4")
            zeros = cst.tile([P, WT], F32, tag="zeros")
            nc.gpsimd.memset(zeros[:], 0.0)
            ones = cst.tile([P, WT], F32, tag="ones")
            nc.gpsimd.memset(ones[:], 1.0)
            bias2 = cst.tile([P, 1], F32, tag="bias2")
            nc.gpsimd.memset(bias2[:], 2.0)
            bias4 = cst.tile([P, 1], F32, tag="bias4")
            nc.gpsimd.memset(bias4[:], 4.0)

            # manual double-buffered one-hot tensors; ohL zero pad written once
            ohL_bufs, ohR_bufs = [], []
            for i in range(2):
                tL = cst.tile([P, XG, ML], BF16, tag=f"ohL{i}")
                nc.gpsimd.memset(tL[:, :, 4 * 24:], 0.0)
                ohL_bufs.append(tL)
                tR = cst.tile([P, XG, NR], BF16, tag=f"ohR{i}")
                ohR_bufs.append(tR)

            slabch_idx = 0

            for f in range(nframes):
                # ---- RGB -> HSV for all slabs of this frame ----
                hsv_tiles = [[None] * 3 for _ in range(NSLAB)]
                for s in range(NSLAB):
                    r0 = s * P
                    rgb = []
                    for c in range(3):
                        t = rgbp.tile([P, WT], F32, tag="rgb")
                        nc.sync.dma_start(out=t[:, 1:513], in_=seq[f, c, r0:r0 + P, :])
                        nc.gpsimd.tensor_copy(out=t[:, 0:1], in_=t[:, 2:3])
                        nc.gpsimd.tensor_copy(out=t[:, 513:514], in_=t[:, 511:512])
                        rgb.append(t)
                    r, g, b = rgb

                    maxc = hsvp.tile([P, WT], F32, tag="hsv")  # = V channel
                    nc.vector.tensor_tensor(out=maxc[:], in0=r[:], in1=g[:], op=AL.max)
                    nc.vector.tensor_tensor(out=maxc[:], in0=maxc[:], in1=b[:], op=AL.max)
                    minc = wrk.tile([P, WT], F32, tag="w")
                    nc.vector.tensor_tensor(out=minc[:], in0=r[:], in1=g[:], op=AL.min)
                    nc.vector.tensor_tensor(out=minc[:], in0=minc[:], in1=b[:], op=AL.min)
                    delta = wrk.tile([P, WT], F32, tag="w")
                    nc.vector.tensor_tensor(out=delta[:], in0=maxc[:], in1=minc[:], op=AL.subtract)
                    mask0 = mskp.tile([P, WT], U8, tag="m")  # delta <= 1e-6
                    nc.vector.tensor_scalar(out=mask0[:], in0=delta[:], scalar1=1e-6,
                                            scalar2=None, op0=AL.is_le)
                    inv = wrk.tile([P, WT], F32, tag="w")
                    nc.vector.select(out=inv[:], mask=mask0[:], on_true=ones[:], on_false=delta[:])
                    nc.vector.reciprocal(out=inv[:], in_=inv[:])

                    hue = hsvp.tile([P, WT], F32, tag="hsv")
                    nc.gpsimd.memset(hue[:], 0.0)
                    tdif = wrk.tile([P, WT], F32, tag="w")
                    tval = wrk.tile([P, WT], F32, tag="w")
                    meq = mskp.tile([P, WT], U8, tag="m")
                    # r-branch: mod((g-b)*inv, 6)
                    nc.vector.tensor_tensor(out=tdif[:], in0=g[:], in1=b[:], op=AL.subtract)
                    nc.vector.tensor_tensor(out=tdif[:], in0=tdif[:], in1=inv[:], op=AL.mult)
                    tneg = wrk.tile([P, WT], F32, tag="w")
                    nc.vector.tensor_scalar(out=tneg[:], in0=tdif[:], scalar1=0.0,
                                            scalar2=None, op0=AL.is_lt)
                    nc.vector.scalar_tensor_tensor(out=tval[:], in0=tneg[:], scalar=6.0,
                                                   in1=tdif[:], op0=AL.mult, op1=AL.add)
                    nc.vector.tensor_tensor(out=meq[:], in0=maxc[:], in1=r[:], op=AL.is_equal)
                    nc.vector.copy_predicated(out=hue[:], mask=meq[:], data=tval[:])
                    # g-branch: (b-r)*inv + 2
                    nc.vector.tensor_tensor(out=tdif[:], in0=b[:], in1=r[:], op=AL.subtract)
                    nc.vector.tensor_tensor(out=tdif[:], in0=tdif[:], in1=inv[:], op=AL.mult)
                    nc.scalar.activation(out=tval[:], in_=tdif[:], func=ACTF.Identity, bias=bias2[:])
                    nc.vector.tensor_tensor(out=meq[:], in0=maxc[:], in1=g[:], op=AL.is_equal)
                    nc.vector.copy_predicated(out=hue[:], mask=meq[:], data=tval[:])
                    # b-branch: (r-g)*inv + 4
                    nc.vector.tensor_tensor(out=tdif[:], in0=r[:], in1=g[:], op=AL.subtract)
                    nc.vector.tensor_tensor(out=tdif[:], in0=tdif[:], in1=inv[:], op=AL.mult)
                    nc.scalar.activation(out=tval[:], in_=tdif[:], func=ACTF.Identity, bias=bias4[:])
                    nc.vector.tensor_tensor(out=meq[:], in0=maxc[:], in1=b[:], op=AL.is_equal)
                    nc.vector.copy_predicated(out=hue[:], mask=meq[:], data=tval[:])
                    # zero where delta<=1e-6, then /6
                    nc.vector.copy_predicated(out=hue[:], mask=mask0[:], data=zeros[:])
                    nc.scalar.activation(out=hue[:], in_=hue[:], func=ACTF.Copy,
                                         scale=float(np.float32(1.0 / 6.0)))

                    # sat
                    sat = hsvp.tile([P, WT], F32, tag="hsv")
                    mx = wrk.tile([P, WT], F32, tag="w")
                    nc.vector.tensor_scalar(out=mx[:], in0=maxc[:], scalar1=1e-6,
                                            scalar2=None, op0=AL.max)
                    nc.vector.reciprocal(out=mx[:], in_=mx[:])
                    nc.vector.tensor_tensor(out=sat[:], in0=delta[:], in1=mx[:], op=AL.mult)
                    msat = mskp.tile([P, WT], U8, tag="m")
                    nc.vector.tensor_scalar(out=msat[:], in0=maxc[:], scalar1=1e-6,
                                            scalar2=None, op0=AL.is_le)
                    nc.vector.copy_predicated(out=sat[:], mask=msat[:], data=zeros[:])

                    hsv_tiles[s] = [hue, sat, maxc]

                # ---- histograms per (channel, slab) ----
                for c in range(3):
                    fc = f * 3 + c
                    pm = ps.tile([P, NR], F32, space="PSUM", tag="pm")
                    for s in range(NSLAB):
                        ct = hsv_tiles[s][c]
                        up = shtp.tile([P, WT], F32, tag="up")
                        if s == 0:
                            nc.sync.dma_start(out=up[0:1], in_=hsv_tiles[0][c][1:2])
                        else:
                            nc.sync.dma_start(out=up[0:1], in_=hsv_tiles[s - 1][c][127:128])
                        nc.sync.dma_start(out=up[1:128], in_=ct[0:127])
                        dn = shtp.tile([P, WT], F32, tag="dn")
                        nc.sync.dma_start(out=dn[0:127], in_=ct[1:128])
                        if s == NSLAB - 1:
                            nc.sync.dma_start(out=dn[127:128], in_=hsv_tiles[s][c][126:127])
                        else:
                            nc.sync.dma_start(out=dn[127:128], in_=hsv_tiles[s + 1][c][0:1])
                        rowt = {-1: up, 0: ct, 1: dn}

                        ohL = ohL_bufs[slabch_idx % 2]
                        ohR = ohR_bufs[slabch_idx % 2]
                        slabch_idx += 1

                        # LBP bits
                        bits = []
                        for k, (dy, dx) in enumerate(LBP_OFFS):
                            bt = bitp.tile([P, W], BF16, tag="bit")
                            eng = nc.vector
                            eng.tensor_tensor(out=bt[:],
                                              in0=rowt[dy][:, 1 + dx:513 + dx],
                                              in1=ct[:, 1:513], op=AL.is_ge)
                            bits.append(bt)
                        lonib = nibp.tile([P, W], BF16, tag="nib")
                        nc.vector.scalar_tensor_tensor(out=lonib[:], in0=bits[1][:], scalar=2.0,
                                                       in1=bits[0][:], op0=AL.mult, op1=AL.add)
                        nc.vector.scalar_tensor_tensor(out=lonib[:], in0=bits[2][:], scalar=4.0,
                                                       in1=lonib[:], op0=AL.mult, op1=AL.add)
                        nc.vector.scalar_tensor_tensor(out=lonib[:], in0=bits[3][:], scalar=8.0,
                                                       in1=lonib[:], op0=AL.mult, op1=AL.add)
                        hinib = nibp.tile([P, W], BF16, tag="nib")
                        nc.vector.scalar_tensor_tensor(out=hinib[:], in0=bits[5][:], scalar=2.0,
                                                       in1=bits[4][:], op0=AL.mult, op1=AL.add)
                        nc.vector.scalar_tensor_tensor(out=hinib[:], in0=bits[6][:], scalar=4.0,
                                                       in1=hinib[:], op0=AL.mult, op1=AL.add)
                        nc.vector.scalar_tensor_tensor(out=hinib[:], in0=bits[7][:], scalar=8.0,
                                                       in1=hinib[:], op0=AL.mult, op1=AL.add)

                        # HSV bin indices with exact floor
                        cm = wrk.tile([P, W], F32, tag="w")
                        nc.vector.tensor_scalar(out=cm[:], in0=ct[:, 1:513], scalar1=C1M,
                                                scalar2=None, op0=AL.min)
                        t32 = wrk.tile([P, W], F32, tag="w")
                        nc.scalar.activation(out=t32[:], in_=cm[:], func=ACTF.Copy, scale=32.0)
                        i32t = wip.tile([P, W], I32, tag="wi")
                        nc.vector.tensor_copy(out=i32t[:], in_=t32[:])
                        f32b = wrk.tile([P, W], F32, tag="w")
                        nc.vector.tensor_copy(out=f32b[:], in_=i32t[:])
                        gt = wrk.tile([P, W], F32, tag="w")
                        nc.vector.tensor_tensor(out=gt[:], in0=f32b[:], in1=t32[:], op=AL.is_gt)
                        idx = wrk.tile([P, W], F32, tag="w")
                        nc.vector.tensor_tensor(out=idx[:], in0=f32b[:], in1=gt[:], op=AL.subtract)
                        h8f = wrk.tile([P, W], F32, tag="w")
                        nc.scalar.activation(out=h8f[:], in_=idx[:], func=ACTF.Copy,
                                             scale=0.25, bias=-0.375)
                        h8i = wip.tile([P, W], I32, tag="wi")
                        nc.vector.tensor_copy(out=h8i[:], in_=h8f[:])
                        h8 = wrk.tile([P, W], F32, tag="w")
                        nc.vector.tensor_copy(out=h8[:], in_=h8i[:])
                        h8b = nibp.tile([P, W], BF16, tag="nib")
                        nc.gpsimd.tensor_copy(out=h8b[:], in_=h8[:])
                        lo4 = nibp.tile([P, W], BF16, tag="nib")
                        nc.vector.scalar_tensor_tensor(out=lo4[:], in0=h8[:], scalar=-4.0,
                                                       in1=idx[:], op0=AL.mult, op1=AL.add)

                        # one-hot writes, interleaved by g = x % G
                        dstL = ohL[:, :, 0:96].rearrange("p x (g t) -> p x g t", g=G)
                        dstR = ohR[:].rearrange("p x (g t) -> p x g t", g=G)
                        io16 = iota16[:].unsqueeze(1).unsqueeze(1).to_broadcast([P, XG, G, 16])
                        io8 = iota8[:].unsqueeze(1).unsqueeze(1).to_broadcast([P, XG, G, 8])
                        io4 = iota4[:].unsqueeze(1).unsqueeze(1).to_broadcast([P, XG, G, 4])

                        src = hinib[:].rearrange("p (x g) -> p x g", g=G).unsqueeze(3)
                        nc.vector.tensor_tensor(out=dstL[:, :, :, 0:16],
                                                in0=src.to_broadcast([P, XG, G, 16]),
                                                in1=io16, op=AL.is_equal)
                        src = h8b[:].rearrange("p (x g) -> p x g", g=G).unsqueeze(3)
                        nc.vector.tensor_tensor(out=dstL[:, :, :, 16:24],
                                                in0=src.to_broadcast([P, XG, G, 8]),
                                                in1=io8, op=AL.is_equal)
                        src = lonib[:].rearrange("p (x g) -> p x g", g=G).unsqueeze(3)
                        nc.vector.tensor_tensor(out=dstR[:, :, :, 0:16],
                                                in0=src.to_broadcast([P, XG, G, 16]),
                                                in1=io16, op=AL.is_equal)
                        src = lo4[:].rearrange("p (x g) -> p x g", g=G).unsqueeze(3)
                        nc.vector.tensor_tensor(out=dstR[:, :, :, 16:20],
                                                in0=src.to_broadcast([P, XG, G, 4]),
                                                in1=io4, op=AL.is_equal)

                        for xg in range(XG):
                            nc.tensor.matmul(
                                out=pm[:],
                                lhsT=ohL[:, xg, :],
                                rhs=ohR[:, xg, :],
                                start=(s == 0 and xg == 0),
                                stop=(s == NSLAB - 1 and xg == XG - 1),
                            )
                    ob = obp.tile([P, NR], F32, tag="ob")
                    nc.vector.tensor_copy(out=ob[:], in_=pm[:])
                    nc.sync.dma_start(out=out[fc], in_=ob[:])

    nc.finalize()
    return nc


_NC_CACHE = {}


def _get_nc(nframes):
    if nframes not in _NC_CACHE:
        _NC_CACHE[nframes] = _build_nc(nframes)
    return _NC_CACHE[nframes]


def device_histograms(frames, nframes_per_core=1, n_cores=8, **run_kwargs):
    """frames: [N, 3, 512, 512] f32. Returns (hsv_counts [N,3,32],
    lbp_counts [N,3,256], BassKernelResults)."""
    nc = _get_nc(nframes_per_core)
    in_maps = []
    for k in range(n_cores):
        sl = frames[k * nframes_per_core:(k + 1) * nframes_per_core]
        in_maps.append({"seq": np.ascontiguousarray(sl, dtype=np.float32)})
    res = run_bass_kernel_spmd(nc, in_maps, core_ids=list(range(n_cores)), **run_kwargs)
    N = nframes_per_core * n_cores
    hsv_counts = np.zeros((N, 3, 32), np.float32)
    lbp_counts = np.zeros((N, 3, 256), np.float32)
    for k in range(n_cores):
        pmall = res.results[k]["out"]
        for f in range(nframes_per_core):
            for c in range(3):
                pm = pmall[f * 3 + c]
                lbp = np.zeros((16, 16), np.float32)
                hsv = np.zeros((8, 4), np.float32)
                for g in range(G):
                    blk = pm[24 * g:24 * g + 24, 20 * g:20 * g + 20]
                    lbp += blk[0:16, 0:16]
                    hsv += blk[16:24, 16:20]
                n = k * nframes_per_core + f
                lbp_counts[n, c] = lbp.reshape(256)
                hsv_counts[n, c] = hsv.reshape(32)
    return hsv_counts, lbp_counts, res


def kernel(sequence, high, W, b, environment_id, season_id):
    sequence = np.asarray(sequence, dtype=np.float32)
    high = np.asarray(high, dtype=np.float32)
    Wp = np.asarray(W, dtype=np.float32)
    bp = np.asarray(b, dtype=np.float32)
    B, T, C, Hh, Ww = sequence.shape
    BT = B * T
    frames = sequence.reshape(BT, C, Hh, Ww)

    npc = 1
    per_batch = npc * 8
    hs, ls = [], []
    for b0 in range(0, BT, per_batch):
        h_, l_, _ = device_histograms(frames[b0:b0 + per_batch], nframes_per_core=npc)
        hs.append(h_)
        ls.append(l_)
    hsv_counts = np.concatenate(hs, axis=0)
    lbp_counts = np.concatenate(ls, axis=0)

    hist_feat = hsv_counts / np.maximum(hsv_counts.sum(-1, keepdims=True), 1.0)
    lbp_feat = lbp_counts / np.maximum(lbp_counts.sum(-1, keepdims=True), 1.0)
    low = np.concatenate([hist_feat.reshape(BT, 96), lbp_feat.reshape(BT, 768)],
                         axis=1).astype(np.float32)
    fused = np.concatenate([high, low], axis=1).reshape(B, T, -1)
    aggregated = fused.mean(axis=1, dtype=np.float32)
    z = (aggregated @ Wp.T + bp).astype(np.float32)
    c_feat = np.where(z >= 0, z, np.float32(0.2) * z).astype(np.float32)

    env_ids = np.asarray(environment_id).astype(np.int64)
    sea_ids = np.asarray(season_id).astype(np.int64)
    env = np.zeros((B, 4), np.float32)
    env[np.arange(B), env_ids] = 1.0
    sea = np.zeros((B, 4), np.float32)
    sea[np.arange(B), sea_ids] = 1.0
    c_cls = np.concatenate([env, sea], axis=1)
    return (c_feat, c_cls)
